# revision 2
# baseline (speedup 1.0000x reference)
"""Trainium2 Bass kernel v3 for nn_CausalSelfAttention (sparse windowed attention).

Tensor-parallel over heads: 8 heads onto 8 cores. Fully transposed dataflow:
the host uploads x already transposed (xT, bf16) and Q/K/V are computed
directly in [head_dim, T] layout by swapping matmul operand roles, so no PE
transposes are needed for Q/K. All DVE/ACT ops keep every operand at the same
start partition (BIR verifier rule); the RoPE pair-swap crosses partitions via
a single PE permutation matmul per tensor-tile, and the result is assembled
with a TT-add drain. Q is RMS-normalized via a row-sum (ones-matmul) ->
reciprocal -> sqrt -> gpsimd partition_broadcast; K stays unnormalized and its
rs_k (pre-multiplied by the softmax scale) folds into the Exp activation's
per-partition scale. V is computed wide in [d, T], PE-transposed per block to
[t, d], with the ve-gate fused into the PSUM drain. Attention uses transposed
scores, post-exp 0/1 bf16 masks, denominator via tiny ones-matmuls, and the
output projection drains with recip*gate fused. bf16 everywhere except PSUM.
"""

import numpy as np
import ml_dtypes
from contextlib import ExitStack

T = 4096
DIM = 1024
H = 8
D = 128
ATTN_SCALE = 0.1
EPS = 1.1920929e-07
QT = 512
NB = T // 128   # 32
NT = T // QT    # 8
BF = ml_dtypes.bfloat16


# ---------------------------------------------------------------- host prep

def _rope_factors():
    n = D // 4
    base = np.float32(1.0 / 1024.0)
    af = base ** np.linspace(0.0, 1.0, n, dtype=np.float32)
    af = np.repeat(af, 2)                       # [64] freqs for dims 0..63
    theta = np.arange(T, dtype=np.float32)[:, None] * af[None, :]   # [T, 64]
    f1 = np.cos(theta).astype(np.float32)       # [T, 64]
    f2 = np.sin(theta).astype(np.float32)
    f2[:, 1::2] *= -1.0                         # f2[:, odd] = -sin
    return f1, f2


def _plan_attention(seqlens, bm):
    """Per q-tile chunk lists with mask ids, + unique 0/1 mask tiles."""
    t = np.arange(T)
    doc = np.searchsorted(seqlens, t, side="right") - 1
    doc_start = np.where(doc >= 0, seqlens[np.clip(doc, 0, len(seqlens) - 1)], 0)
    lo = np.maximum(np.maximum(t - bm, doc_start), 0)

    masks = {}
    plan = []
    for j in range(NT):
        q = np.arange(j * QT, (j + 1) * QT)
        lo_q = lo[q]
        entries = []
        for kc in range(NB):
            k = np.arange(kc * 128, kc * 128 + 128)
            M = (k[:, None] <= q[None, :]) & (k[:, None] >= lo_q[None, :])
            if not M.any():
                continue
            if M.all():
                mid = None
            else:
                key = M.tobytes()
                if key not in masks:
                    masks[key] = (len(masks), M.astype(np.float32))
                mid = masks[key][0]
            sub_any = tuple(bool(M[:, s * 128:(s + 1) * 128].any()) for s in range(4))
            cols = M.any(axis=0)
            lo_c = int(np.argmax(cols))
            hi_c = int(QT - np.argmax(cols[::-1]))
            entries.append((kc, mid, sub_any, lo_c, hi_c))
        # first entry must span all columns (diagonal coverage) for psum start
        assert entries[0][3] == 0 and entries[0][4] == QT, entries[0]
        plan.append(entries)
    n_masks = max(len(masks), 1)
    arr = np.zeros((128, n_masks, QT), np.float32)
    for _, (mid, m) in masks.items():
        arr[:, mid, :] = m
    return plan, arr, n_masks


def _chunked(w):
    """[DIM, 128] -> [128, 8, 128] d-chunk-major bf16."""
    return np.ascontiguousarray(w.reshape(8, 128, 128).transpose(1, 0, 2)).astype(BF)


def _pack_head(h, x2, ve, wqkv, wo_full, attn_gate_w, ve_gate_w, mask01):
    """Per-core input map (bf16 unless noted). Natural head-dim order."""
    hs = slice(h * D, (h + 1) * D)
    Wq = wqkv[0 * DIM:1 * DIM][hs]      # [128, DIM] rows = q dims
    Wk = wqkv[1 * DIM:2 * DIM][hs]
    Wv = wqkv[2 * DIM:3 * DIM][hs]

    # gate weights [18, 2]: col0 ve gate, col1 attn gate
    gw = np.zeros((18, 2), np.float32)
    gw[0:6, 0] = ve_gate_w[h, 0:6]
    gw[12:18, 0] = ve_gate_w[h, 6:12]
    gw[0:12, 1] = attn_gate_w[h, :]

    f1, f2 = _rope_factors()                    # [T, 64]
    f1p = np.ones((128, T), np.float32)         # rows 64:128 passthrough = 1
    f2sw = np.zeros((128, T), np.float32)       # rows 64:128 = 0
    f1p[0:64] = f1.T
    flip = np.arange(64) ^ 1
    f2sw[0:64] = f2.T[flip]                     # row r holds f2[flip(r)]

    pswap = np.zeros((128, 128), np.float32)    # out[p] = in[p^1]
    idx = np.arange(128)
    pswap[idx ^ 1, idx] = 1.0

    estack = np.zeros((1, 8 * 8), np.float32)   # lhsT [1,8] blocks
    for r in range(8):
        estack[0, 8 * r + r] = 1.0
    sel8 = np.zeros((8, 8, 128), np.float32)    # bcast lhsT: block r row r ones
    for r in range(8):
        sel8[r, r, :] = 1.0

    return {
        "xT8": x2,                                          # [128, 8, T] bf16 (shared)
        "ve6T": np.ascontiguousarray(ve[:, :6].T).astype(BF),
        "wqT": _chunked(Wq.T),
        "wkT": _chunked(Wk.T),
        "wvT": _chunked(Wv.T),
        "gw": gw.astype(BF),
        "f1p": f1p.astype(BF),
        "f2sw": f2sw.astype(BF),
        "pswap": pswap.astype(BF),
        "estack": estack.astype(BF),
        "sel8": np.ascontiguousarray(sel8).astype(BF),
        "ones2": np.ones((128, 2), np.float32).astype(BF),
        "identb": np.eye(128, dtype=np.float32).astype(BF),
        "veh2": np.ascontiguousarray(2.0 * ve[:, hs]).astype(BF),
        "masks01": mask01.astype(BF),                        # [128, nmask, QT]
        "wo": np.ascontiguousarray(wo_full[:, hs].T).astype(BF),  # [128, DIM]
    }


def _prep_all(inputs):
    x = np.asarray(inputs["x"], np.float32).reshape(T, DIM)
    ve = np.asarray(inputs["ve"], np.float32)
    qkvo_w = np.asarray(inputs["qkvo_w"], np.float32)
    sa = np.asarray(inputs["sa_lambdas"], np.float32)
    attn_gate_w = np.asarray(inputs["attn_gate_w"], np.float32)
    ve_gate_w = np.asarray(inputs["ve_gate_w"], np.float32)
    seqlens = np.asarray(inputs["seqlens"])
    bm = int(np.asarray(inputs["bm_size"]))
    key_offset = int(np.asarray(inputs["key_offset"]))

    wqkv = (sa[0] * qkvo_w[:3 * DIM]).astype(np.float32)
    wo_full = (sa[1] * qkvo_w[3 * DIM:]).astype(np.float32)    # [DIM, DIM]

    plan, mask01, n_masks = _plan_attention(seqlens, bm)

    xT = x.T                                                    # [DIM, T]
    x2 = np.ascontiguousarray(
        xT.reshape(8, 128, T).transpose(1, 0, 2)).astype(BF)    # [128, 8, T]

    in_maps = [
        _pack_head(h, x2, ve, wqkv, wo_full, attn_gate_w, ve_gate_w, mask01)
        for h in range(H)
    ]
    return in_maps, plan, n_masks, key_offset, seqlens, bm


# ------------------------------------------------------------ numpy emulation

def emulate(**inputs):
    """Numpy emulation of the exact device dataflow (f32 math, bf16 data)."""
    in_maps, plan, n_masks, key_offset, _, _ = _prep_all(inputs)
    bfr = lambda a: a.astype(BF).astype(np.float32)
    out = np.zeros((T, DIM), np.float64)
    for h in range(H):
        m = {k: np.asarray(v, np.float32) for k, v in in_maps[h].items()}
        xT8, wqT, wkT, wvT = m["xT8"], m["wqT"], m["wkT"], m["wvT"]
        f1p, f2sw, pswap = m["f1p"], m["f2sw"], m["pswap"]
        veh2, gw, ve6T, wo = m["veh2"], m["gw"], m["ve6T"], m["wo"]
        masks01 = m["masks01"]

        # gates
        feat = np.concatenate([xT8[0:12, 0, :], ve6T], axis=0)   # [18, T]
        gsig = 1.0 / (1.0 + np.exp(-(feat.T @ gw)))              # [T, 2]

        # wide passes
        def wide(wc):
            r = np.zeros((128, T), np.float32)
            for c in range(8):
                r += wc[:, c, :].T @ xT8[:, c, :]
            return bfr(r)
        rawq, rawk, vT = wide(wqT), wide(wkT), wide(wvT)

        sq_q, sq_k = bfr(rawq * rawq), bfr(rawk * rawk)
        # q/k sums: per-tile row sums -> stack interleaved -> rs_qk [16, 512]
        smq = bfr(sq_q.sum(axis=0).reshape(NT, QT))              # [8, 512]
        smk = bfr(sq_k.sum(axis=0).reshape(NT, QT))
        sm = np.empty((2 * NT, QT), np.float32)
        sm[0::2], sm[1::2] = smq, smk
        rs_qk = bfr(np.sqrt(1.0 / (sm / D + EPS)))               # [16, 512] bf16

        qf = np.zeros((128, T), np.float32)
        kf = np.zeros((128, T), np.float32)
        for i in range(NT):
            ts = slice(QT * i, QT * (i + 1))
            qn = bfr(rawq[:, ts] * np.broadcast_to(rs_qk[2 * i], (128, QT)))
            kn = bfr(rawk[:, ts] * np.broadcast_to(rs_qk[2 * i + 1], (128, QT)))
            m1q, m2q = bfr(qn * f1p[:, ts]), bfr(qn * f2sw[:, ts])
            m1k = bfr(kn * f1p[:, ts])
            m2k = bfr(kn * f2sw[:, ts])
            qf[:, ts] = bfr(m1q + pswap.T @ m2q)
            kfl = m1k + pswap.T @ m2k
            kf[0:64, ts] = bfr(kfl[0:64])
            if key_offset:
                lo_ = QT * i + 1
                hi = min(QT * (i + 1) + 1, T)
                kf[64:128, lo_:hi] = bfr(kfl[64:128, 0:hi - lo_])
                if i == 0:
                    kf[64:128, 0] = bfr(kfl[64:128, 0])
            else:
                kf[64:128, ts] = bfr(kfl[64:128])

        # v blocks + ve gating
        v_sb = np.zeros((NB, 128, 128), np.float32)              # [b, t_loc, d]
        for b in range(NB):
            bc = slice(128 * b, 128 * (b + 1))
            v_sb[b] = bfr(vT[:, bc].T + gsig[bc, 0:1] * veh2[bc])

        # attention
        yT = np.zeros((128, T), np.float32)
        den = np.zeros(T, np.float32)
        for j in range(NT):
            qs = slice(QT * j, QT * (j + 1))
            for (kc, mid, _sub, lo_c, hi_c) in plan[j]:
                kcc = slice(128 * kc, 128 * (kc + 1))
                sp = kf[:, kcc].T @ qf[:, qs]                    # [128k, 512q]
                p = bfr(np.exp(ATTN_SCALE * sp))
                if mid is not None:
                    p = bfr(p * masks01[:, mid, :])
                yT[:, qs] += v_sb[kc].T @ p
                den[qs] += p.sum(axis=0)
        yT = bfr(yT)

        sfin = (1.0 / den) * gsig[:, 1]
        for b in range(NB):
            bc = slice(128 * b, 128 * (b + 1))
            ob = ((yT[:, bc].T @ wo) * sfin[bc, None]).astype(BF)
            out[bc] += np.asarray(ob, np.float32)
    return out.astype(np.float32).reshape(1, T, DIM)


# ---------------------------------------------------------------- bass build

def _build(plan, n_masks, key_offset):
    import concourse.bass as bass
    import concourse.bacc as bacc
    import concourse.mybir as mybir
    from concourse import tile

    dt = mybir.dt
    f32, bf16 = dt.float32, dt.bfloat16
    OP = mybir.AluOpType
    AF = mybir.ActivationFunctionType

    nc = bacc.Bacc()
    xT8_d = nc.dram_tensor("xT8", [128, 8, T], bf16, kind="ExternalInput").ap()
    ve6T_d = nc.dram_tensor("ve6T", [6, T], bf16, kind="ExternalInput").ap()
    wqT_d = nc.dram_tensor("wqT", [128, 8, 128], bf16, kind="ExternalInput").ap()
    wkT_d = nc.dram_tensor("wkT", [128, 8, 128], bf16, kind="ExternalInput").ap()
    wvT_d = nc.dram_tensor("wvT", [128, 8, 128], bf16, kind="ExternalInput").ap()
    gw_d = nc.dram_tensor("gw", [18, 2], bf16, kind="ExternalInput").ap()
    f1p_d = nc.dram_tensor("f1p", [128, T], bf16, kind="ExternalInput").ap()
    f2sw_d = nc.dram_tensor("f2sw", [128, T], bf16, kind="ExternalInput").ap()
    pswap_d = nc.dram_tensor("pswap", [128, 128], bf16, kind="ExternalInput").ap()
    estack_d = nc.dram_tensor("estack", [1, 64], bf16, kind="ExternalInput").ap()
    sel8_d = nc.dram_tensor("sel8", [8, 8, 128], bf16, kind="ExternalInput").ap()
    ones2_d = nc.dram_tensor("ones2", [128, 2], bf16, kind="ExternalInput").ap()
    identb_d = nc.dram_tensor("identb", [128, 128], bf16, kind="ExternalInput").ap()
    veh2_d = nc.dram_tensor("veh2", [T, 128], bf16, kind="ExternalInput").ap()
    masks_d = nc.dram_tensor("masks01", [128, n_masks, QT], bf16, kind="ExternalInput").ap()
    wo_d = nc.dram_tensor("wo", [128, DIM], bf16, kind="ExternalInput").ap()
    out_d = nc.dram_tensor("out", [T, DIM], bf16, kind="ExternalOutput").ap()

    with ExitStack() as ctx:
        tc = ctx.enter_context(tile.TileContext(nc))
        consts = ctx.enter_context(tc.tile_pool(name="consts", bufs=1))
        state = ctx.enter_context(tc.tile_pool(name="state", bufs=1))

        # ---- tiny consts needed first (gates + tile-0 path) ----
        feat = consts.tile([18, T], bf16)
        nc.sync.dma_start(feat[0:12, :], xT8_d[0:12, 0, :])
        nc.scalar.dma_start(feat[12:18, :], ve6T_d[:])
        gw = consts.tile([18, 2], bf16)
        nc.sync.dma_start(gw[:], gw_d[:])
        ones2 = consts.tile([128, 2], bf16)
        nc.scalar.dma_start(ones2[:], ones2_d[:])
        identb = consts.tile([128, 128], bf16)
        nc.sync.dma_start(identb[:], identb_d[:])
        wq = consts.tile([128, 8 * 128], bf16)
        nc.sync.dma_start(wq[:].rearrange("p (c n) -> p c n", n=128), wqT_d[:])
        wk = consts.tile([128, 8 * 128], bf16)
        nc.scalar.dma_start(wk[:].rearrange("p (c n) -> p c n", n=128), wkT_d[:])
        wv = consts.tile([128, 8 * 128], bf16)
        nc.sync.dma_start(wv[:].rearrange("p (c n) -> p c n", n=128), wvT_d[:])
        pswap = consts.tile([128, 128], bf16)
        nc.scalar.dma_start(pswap[:], pswap_d[:])
        estack = consts.tile([1, 64], bf16)
        nc.sync.dma_start(estack[:], estack_d[:])
        sel8 = consts.tile([8, 8 * 128], bf16)
        nc.scalar.dma_start(sel8[:].rearrange("p (c n) -> p c n", n=128), sel8_d[:])
        f1p = consts.tile([128, T], bf16)
        f2sw = consts.tile([128, T], bf16)
        msk = consts.tile([128, n_masks * QT], bf16)
        veh = consts.tile([128, NB * 128], bf16)
        wo = consts.tile([128, DIM], bf16)

        # ---- state ----
        rawq = state.tile([128, T], bf16)
        rawk = state.tile([128, T], bf16)
        qf = state.tile([128, T], bf16)
        kf = state.tile([128, T], bf16)
        v_sb = state.tile([128, T], bf16)
        yT = state.tile([128, T], bf16)
        gates_sig = state.tile([128, 2 * NB], f32)
        smq_sb = state.tile([1, NT * QT], bf16)
        smk_sb = state.tile([1, NT * QT], bf16)
        mqk = state.tile([8, QT], f32)
        rs_h0 = state.tile([8, QT], bf16)
        rs_h1 = state.tile([8, QT], bf16)
        recip_t = state.tile([128, NB], f32)
        sfin = state.tile([128, NB], f32)

        gsig3 = gates_sig[:].rearrange("p (n w) -> p n w", w=2)

        xcp = ctx.enter_context(tc.tile_pool(name="xcp", bufs=1))
        scr = ctx.enter_context(tc.tile_pool(name="scr", bufs=3))
        ropep = ctx.enter_context(tc.tile_pool(name="rop", bufs=2))
        vtpool = ctx.enter_context(tc.tile_pool(name="vts", bufs=2))
        ppool = ctx.enter_context(tc.tile_pool(name="pch", bufs=3))
        opool = ctx.enter_context(tc.tile_pool(name="ob", bufs=2))

        veh_src = veh2_d[:].rearrange("(b p) d -> p b d", p=128)
        veh_dst = veh[:].rearrange("p (b d) -> p b d", d=128)
        # xT full chunk tiles, piece-major upfront DMA
        xc = []
        for c in range(8):
            xt_ = xcp.tile([128, T], bf16, tag=f"xc{c}")
            xc.append(xt_)
        for i in range(8):
            for c in range(8):
                eng = nc.sync if (i * 8 + c) % 2 == 0 else nc.scalar
                eng.dma_start(xc[c][:, QT * i:QT * (i + 1)],
                              xT8_d[:, c, QT * i:QT * (i + 1)])
            if i % 2 == 1:
                v4 = (i // 2) * 8
                nc.sync.dma_start(veh_dst[:, v4:v4 + 8, :],
                                  veh_src[:, v4:v4 + 8, :])

        pools = {}
        with tc.tile_pool(name="mm", bufs=3, space="PSUM") as mmp, \
             tc.tile_pool(name="vtp", bufs=1, space="PSUM") as vtpsum:

            def gates():
                gpsum = pools["sm"].tile([128, 64], f32, tag="sm")
                for b in range(NB):
                    bc = slice(128 * b, 128 * (b + 1))
                    nc.tensor.matmul(gpsum[:, 2 * b:2 * b + 2], lhsT=feat[:, bc],
                                     rhs=gw[:], start=True, stop=True,
                                     skip_group_check=True)
                nc.scalar.activation(gates_sig[:], gpsum[:], AF.Sigmoid)

            def wide_tile(i):
                ts = slice(QT * i, QT * (i + 1))
                qp = mmp.tile([128, QT], f32, tag="mm")
                for c in range(8):
                    nc.tensor.matmul(qp[:], lhsT=wq[:, 128 * c:128 * (c + 1)],
                                     rhs=xc[c][:, ts], start=(c == 0), stop=(c == 7))
                nc.scalar.copy(rawq[:, ts], qp[:])
                sqq = scr.tile([128, QT], bf16, tag="sqq")
                nc.vector.tensor_mul(sqq[:], rawq[:, ts], rawq[:, ts])
                smpq = pools["sm"].tile([1, QT], f32, tag="smq")
                nc.tensor.matmul(smpq[:], lhsT=ones2[:, 0:1], rhs=sqq[:],
                                 start=True, stop=True)
                nc.scalar.copy(smq_sb[:, ts], smpq[:])
                kp = mmp.tile([128, QT], f32, tag="mm")
                for c in range(8):
                    nc.tensor.matmul(kp[:], lhsT=wk[:, 128 * c:128 * (c + 1)],
                                     rhs=xc[c][:, ts], start=(c == 0), stop=(c == 7))
                nc.scalar.copy(rawk[:, ts], kp[:])
                sqk = scr.tile([128, QT], bf16, tag="sqk")
                nc.vector.tensor_mul(sqk[:], rawk[:, ts], rawk[:, ts])
                smpk = pools["sm"].tile([1, QT], f32, tag="smq")
                nc.tensor.matmul(smpk[:], lhsT=ones2[:, 0:1], rhs=sqk[:],
                                 start=True, stop=True)
                nc.scalar.copy(smk_sb[:, ts], smpk[:])
                vp = mmp.tile([128, QT], f32, tag="mm")
                for c in range(8):
                    nc.tensor.matmul(vp[:], lhsT=wv[:, 128 * c:128 * (c + 1)],
                                     rhs=xc[c][:, ts], start=(c == 0), stop=(c == 7))
                vt_ = vtpool.tile([128, QT], bf16, tag="vt")
                nc.vector.tensor_copy(vt_[:], vp[:])
                for s_ in range(4):
                    b = 4 * i + s_
                    bc = slice(128 * b, 128 * (b + 1))
                    vtr = vtpsum.tile([128, 128], bf16, tag="vtr")
                    nc.tensor.transpose(vtr[:], vt_[:, 128 * s_:128 * (s_ + 1)],
                                        identb[:])
                    nc.vector.scalar_tensor_tensor(
                        out=v_sb[:, bc], in0=veh[:, bc],
                        scalar=gates_sig[:, 2 * b:2 * b + 1], in1=vtr[:],
                        op0=OP.mult, op1=OP.add)

            def rms_half(h_, rs_h):
                stk = pools["stk"].tile([8, QT], f32, tag="stk")
                for r in range(8):
                    i_ = 4 * h_ + r // 2
                    src = smq_sb if r % 2 == 0 else smk_sb
                    nc.tensor.matmul(stk[:], lhsT=estack[:, 8 * r:8 * (r + 1)],
                                     rhs=src[:, QT * i_:QT * (i_ + 1)],
                                     start=(r == 0), stop=(r == 7))
                nc.vector.tensor_scalar(out=mqk[:], in0=stk[:], scalar1=1.0 / D,
                                        scalar2=EPS, op0=OP.mult, op1=OP.add)
                nc.vector.reciprocal(mqk[:], mqk[:])
                nc.scalar.activation(rs_h[:], mqk[:], AF.Sqrt)

            def rope_tile(i):
                ts = slice(QT * i, QT * (i + 1))
                rs_h = rs_h0 if i < 4 else rs_h1
                rq, rk = 2 * (i % 4), 2 * (i % 4) + 1
                bq = mmp.tile([128, QT], f32, tag="mm")
                nc.tensor.matmul(bq[:], lhsT=sel8[:, 128 * rq:128 * (rq + 1)],
                                 rhs=rs_h[:], start=True, stop=True)
                rsBq = scr.tile([128, QT], bf16, tag="rsbq")
                nc.scalar.copy(rsBq[:], bq[:])
                bk = mmp.tile([128, QT], f32, tag="mm")
                nc.tensor.matmul(bk[:], lhsT=sel8[:, 128 * rk:128 * (rk + 1)],
                                 rhs=rs_h[:], start=True, stop=True)
                rsBk = scr.tile([128, QT], bf16, tag="rsbk")
                nc.vector.tensor_copy(rsBk[:], bk[:])
                qn = ropep.tile([128, QT], bf16, tag="n")
                nc.vector.tensor_mul(qn[:], rawq[:, ts], rsBq[:])
                kn = ropep.tile([128, QT], bf16, tag="n")
                nc.vector.tensor_mul(kn[:], rawk[:, ts], rsBk[:])
                m1q = ropep.tile([128, QT], bf16, tag="m1")
                nc.vector.tensor_mul(m1q[:], qn[:], f1p[:, ts])
                m2q = ropep.tile([128, QT], bf16, tag="m2")
                nc.vector.tensor_mul(m2q[:], qn[:], f2sw[:, ts])
                sw = mmp.tile([128, QT], f32, tag="mm")
                nc.tensor.matmul(sw[:], lhsT=pswap[:], rhs=m2q[:],
                                 start=True, stop=True)
                nc.vector.tensor_add(qf[:, ts], m1q[:], sw[:])
                m1k = ropep.tile([128, QT], bf16, tag="m1")
                nc.vector.tensor_mul(m1k[:], kn[:], f1p[:, ts])
                m2k = ropep.tile([128, QT], bf16, tag="m2")
                nc.vector.tensor_mul(m2k[:], kn[:], f2sw[:, ts])
                swk = mmp.tile([128, QT], f32, tag="mm")
                nc.tensor.matmul(swk[:], lhsT=pswap[:], rhs=m2k[:],
                                 start=True, stop=True)
                nc.vector.tensor_add(kf[0:64, ts], m1k[0:64, :], swk[0:64, :])
                if key_offset:
                    if i == 0:
                        nc.vector.tensor_add(kf[64:128, 0:1], m1k[64:128, 0:1],
                                             swk[64:128, 0:1])
                    w_ = QT if i < NT - 1 else QT - 1
                    nc.vector.tensor_add(kf[64:128, QT * i + 1:QT * i + 1 + w_],
                                         m1k[64:128, 0:w_], swk[64:128, 0:w_])
                else:
                    nc.vector.tensor_add(kf[64:128, ts], m1k[64:128, :],
                                         swk[64:128, :])

            def attn_tile(j):
                qs = slice(QT * j, QT * (j + 1))
                entries = plan[j]
                y_ps = mmp.tile([128, QT], f32, tag="mm")
                den_ps = pools["den"].tile([128, 8], f32, tag="den")
                den_pairs = [(kc, s) for kc, _, sa, _, _ in entries
                             for s in range(4) if sa[s]]
                den_first, den_last = den_pairs[0], den_pairs[-1]

                def pv_and_den(ei, kc, sub_any, lo_c, hi_c, p_sb):
                    kcc = slice(128 * kc, 128 * (kc + 1))
                    nc.tensor.matmul(y_ps[:, lo_c:hi_c], lhsT=v_sb[:, kcc],
                                     rhs=p_sb[:, lo_c:hi_c],
                                     start=(ei == 0), stop=(ei == len(entries) - 1),
                                     skip_group_check=True)
                    for s in range(4):
                        if sub_any[s]:
                            nc.tensor.matmul(den_ps[:, 2 * s:2 * s + 2],
                                             lhsT=p_sb[:, 128 * s:128 * (s + 1)],
                                             rhs=ones2[:],
                                             start=((kc, s) == den_first),
                                             stop=((kc, s) == den_last),
                                             skip_group_check=True)

                pend_pv = []
                for ei, (kc, mid, sub_any, lo_c, hi_c) in enumerate(entries):
                    kcc = slice(128 * kc, 128 * (kc + 1))
                    sp = mmp.tile([128, QT], f32, tag="mm")
                    nc.tensor.matmul(sp[:, lo_c:hi_c], lhsT=kf[:, kcc],
                                     rhs=qf[:, QT * j + lo_c:QT * j + hi_c],
                                     start=True, stop=True)
                    p_sb = ppool.tile([128, QT], bf16, tag="p")
                    if lo_c > 0:
                        nc.gpsimd.memset(p_sb[:, 0:lo_c], 0.0)
                    if hi_c < QT:
                        nc.gpsimd.memset(p_sb[:, hi_c:QT], 0.0)
                    nc.scalar.activation(p_sb[:, lo_c:hi_c], sp[:, lo_c:hi_c],
                                         AF.Exp, scale=ATTN_SCALE)
                    if mid is not None:
                        nc.vector.tensor_mul(
                            p_sb[:, lo_c:hi_c], p_sb[:, lo_c:hi_c],
                            msk[:, QT * mid + lo_c:QT * mid + hi_c])
                    pend_pv.append((ei, kc, sub_any, lo_c, hi_c, p_sb))
                    if len(pend_pv) >= 2:
                        pv_and_den(*pend_pv.pop(0))
                while pend_pv:
                    pv_and_den(*pend_pv.pop(0))

                den3 = den_ps[:].rearrange("p (s w) -> p s w", w=2)
                nc.vector.reciprocal(recip_t[:, 4 * j:4 * j + 4], den3[:, :, 0])
                nc.vector.tensor_tensor(sfin[:, 4 * j:4 * j + 4],
                                        recip_t[:, 4 * j:4 * j + 4],
                                        gsig3[:, 4 * j:4 * j + 4, 1], op=OP.mult)
                nc.vector.tensor_copy(yT[:, qs], y_ps[:])
                for s in range(4):
                    b = 4 * j + s
                    bc = slice(128 * b, 128 * (b + 1))
                    op_ps = pools["op"].tile([128, DIM], f32, tag="o")
                    for hh in range(2):
                        nc.tensor.matmul(op_ps[:, 512 * hh:512 * (hh + 1)],
                                         lhsT=yT[:, bc],
                                         rhs=wo[:, 512 * hh:512 * (hh + 1)],
                                         start=True, stop=True)
                    ob = opool.tile([128, DIM], bf16)
                    if s % 2 == 0:
                        nc.vector.tensor_scalar_mul(ob[:], op_ps[:], sfin[:, b:b + 1])
                    else:
                        nc.scalar.mul(ob[:], op_ps[:], sfin[:, b:b + 1])
                    nc.gpsimd.dma_start(out_d[bc, :], ob[:])

            # ---------------- schedule ----------------
            with tc.tile_pool(name="sm", bufs=1, space="PSUM") as smp_, \
                 tc.tile_pool(name="stk", bufs=1, space="PSUM") as stkp_:
                pools["sm"] = smp_
                pools["stk"] = stkp_
                gates()
                for i in range(4):
                    wide_tile(i)
                    if i == 1:
                        for i2 in range(4):
                            cs = slice(1024 * i2, 1024 * (i2 + 1))
                            eng = nc.sync if i2 % 2 == 0 else nc.scalar
                            eng.dma_start(f1p[:, cs], f1p_d[:, cs])
                            eng.dma_start(f2sw[:, cs], f2sw_d[:, cs])
                    if i == 2:
                        nc.sync.dma_start(
                            msk[:].rearrange("p (c n) -> p c n", n=QT), masks_d[:])
                        nc.scalar.dma_start(wo[:], wo_d[:])
                rms_half(0, rs_h0)
                for i in range(4, 8):
                    wide_tile(i)
                    rope_tile(i - 4)
                rms_half(1, rs_h1)
            with tc.tile_pool(name="dp", bufs=1, space="PSUM") as dpp_, \
                 tc.tile_pool(name="op", bufs=1, space="PSUM") as opp_:
                pools["den"] = dpp_
                pools["op"] = opp_
                attn_tile(0)
                rope_tile(4)
                attn_tile(1)
                rope_tile(5)
                attn_tile(2)
                rope_tile(6)
                attn_tile(3)
                rope_tile(7)
                for j in range(4, 8):
                    attn_tile(j)
    nc.finalize()
    return nc


_CACHE = {}


def _get_program(seqlens, bm, key_offset):
    key = (seqlens.tobytes(), int(bm), int(key_offset))
    if key not in _CACHE:
        plan, mask01, n_masks = _plan_attention(seqlens, bm)
        nc = _build(plan, n_masks, key_offset)
        _CACHE[key] = nc
    return _CACHE[key]


def _run(inputs, trace=False):
    from concourse.bass_utils import run_bass_kernel_spmd

    in_maps, plan, n_masks, key_offset, seqlens, bm = _prep_all(inputs)
    nc = _get_program(np.asarray(seqlens), bm, key_offset)
    res = run_bass_kernel_spmd(nc, in_maps, core_ids=list(range(H)), trace=trace)
    out = np.zeros((T, DIM), np.float32)
    for r in res.results:
        out += np.asarray(r["out"], np.float32)
    return out.reshape(1, T, DIM), res


def kernel(**inputs) -> np.ndarray:
    out, _ = _run(inputs, trace=False)
    return out
